# revision 45
# baseline (speedup 1.0000x reference)
"""Multi-head attention (degenerate multiplicative-mask softmax) on 8 TRN2 cores.

Sharding: pure data-parallel over batch (B=8 -> 1 batch element per core).
No collectives. Each core computes its batch's full attention + output proj.

Structure (vs the 761us baseline; now fused-pipeline):
  - Projection and attention are INTERLEAVED per head-pair: iteration r
    projects q/k columns for head pair r while running the softmax-select
    pipeline (DVE/scalar/gpsimd) of head pair r-1. TensorE stays busy with
    proj matmuls while the other engines chew on selection, instead of two
    serial phases. q/k live in a rolling 3-deep pool (saves 6.5MB SBUF so
    x can stay resident).
  - QK^T: packed hi/lo 2-pass: contraction holds [qh(64); ql(64)] against
    [kh; kh] then [kl; kl] -> exact 4-term (qh+ql)(kh+kl) in 2 full-util
    matmuls. Operands DMA-staged per head (shift/duplication for free).
  - Mask-multiply split: half on DVE straight from PSUM, half via scalar
    drain + gpsimd tensor_tensor on a host-prescaled {0, 1.25e8} bf16 mask.
    DVE does the row-min reduce; exp(umin - u) on scalar; P@V drains on DVE.
    (tensor_tensor_reduce would fuse mask+min but crashes the device -
    NRT_EXEC_UNIT_UNRECOVERABLE, verified in isolation. gpsimd tensor_scalar
    runs in ~15us slow ucode; only plain tensor_tensor is fast there.)
  - Bias matmuls elided when biases are all-zero (the spec fills zeros);
    generic all-bias path kept as a fallback build.
Precision (validated in CPU sim vs f32 reference: 2 argmin flips/131072,
rel err 0.006): 3-pass bf16 hi/lo projections, exact packed QK^T on the
bf16-pair q/k, single-pass bf16 V/output path.
"""
import sys

sys.path.insert(0, "/opt/trn_rl_repo")

import numpy as np
import ml_dtypes

import concourse.bass as bass
import concourse.tile as tile
from concourse import bacc, mybir
from concourse.bass_utils import run_bass_kernel_spmd

F32 = mybir.dt.float32
BF16 = mybir.dt.bfloat16
MULT = mybir.AluOpType.mult
MIN = mybir.AluOpType.min

B, S, D = 8, 1024, 1024
H, DH = 16, 64
P = 128
NT = S // P
SCALE = 1.25e8  # 1e9 / 8

_CACHE = {}


def _bf16(a):
    return np.ascontiguousarray(a.astype(ml_dtypes.bfloat16))


def _build(with_bias=False):
    nc = bacc.Bacc(None)

    xh_d = nc.dram_tensor("xh", [D, S], BF16, kind="ExternalInput")  # x[b].T hi
    xl_d = nc.dram_tensor("xl", [D, S], BF16, kind="ExternalInput")  # x[b].T lo
    # mask pre-scaled by host to {0, SCALE} (both exact in bf16)
    m_d = nc.dram_tensor("m", [S, S], BF16, kind="ExternalInput")
    wqkh_d = nc.dram_tensor("wqkh", [D, 2 * D], BF16, kind="ExternalInput")
    wqkl_d = nc.dram_tensor("wqkl", [D, 2 * D], BF16, kind="ExternalInput")
    wv_d = nc.dram_tensor("wv", [D, D], BF16, kind="ExternalInput")
    wp_d = nc.dram_tensor("wp", [D, D], BF16, kind="ExternalInput")
    id_d = nc.dram_tensor("ident", [P, P], BF16, kind="ExternalInput")
    y_d = nc.dram_tensor("y", [S, D], F32, kind="ExternalOutput")
    yp_d = nc.dram_tensor("yp", [S, D], F32, kind="Internal")  # outproj partial
    if with_bias:
        bqkh_d = nc.dram_tensor("bqkh", [1, 2 * D], BF16, kind="ExternalInput")
        bqkl_d = nc.dram_tensor("bqkl", [1, 2 * D], BF16, kind="ExternalInput")
        bvh_d = nc.dram_tensor("bvh", [1, D], BF16, kind="ExternalInput")
        bvl_d = nc.dram_tensor("bvl", [1, D], BF16, kind="ExternalInput")
        bph_d = nc.dram_tensor("bph", [1, D], BF16, kind="ExternalInput")
        bpl_d = nc.dram_tensor("bpl", [1, D], BF16, kind="ExternalInput")

    with tile.TileContext(nc) as tc:
        with (
            tc.tile_pool(name="res", bufs=1) as res,
            tc.tile_pool(name="vres", bufs=1) as vres,
        ):
            mscl = res.tile([P, NT, S], BF16, tag="mscl")  # [i_sub, i_tile, j]
            otm = res.tile([P, NT, S], BF16, tag="otm")  # OT: [o_sub, o_tile, s]
            ident = res.tile([P, P], BF16, tag="ident")
            biases = {}
            if with_bias:
                ones_row = res.tile([1, S], BF16, tag="ones")
                nc.vector.memset(ones_row[:], 1.0)
                for nm, dd in (("bqkh", bqkh_d), ("bqkl", bqkl_d), ("bvh", bvh_d),
                               ("bvl", bvl_d), ("bph", bph_d), ("bpl", bpl_d)):
                    t = res.tile([1, dd.shape[1]], BF16, tag=nm)
                    nc.sync.dma_start(t[:], dd[:])
                    biases[nm] = t
            vmat = vres.tile([P, NT, D], BF16, tag="vmat")  # [j_sub, j_tile, c]

            # ------------- fused projection + attention -------------
            with tc.tile_pool(name="p12", bufs=1) as p12, \
                 tc.tile_pool(name="wstr", bufs=3) as wstr, \
                 tc.tile_pool(name="qkroll", bufs=2) as qkroll, \
                 tc.tile_pool(name="stg", bufs=3) as stg, \
                 tc.tile_pool(name="ppool", bufs=3) as ppool, \
                 tc.tile_pool(name="ptpool", bufs=2) as ptpool, \
                 tc.tile_pool(name="psA", bufs=2, space="PSUM") as psA, \
                 tc.tile_pool(name="ps_s", bufs=2, space="PSUM") as ps_s, \
                 tc.tile_pool(name="ps_tr", bufs=2, space="PSUM") as ps_tr, \
                 tc.tile_pool(name="ps_o", bufs=2, space="PSUM") as ps_o:
                xh = p12.tile([P, NT, S], BF16, tag="xh")  # [d_sub, d_tile, s]
                xl = p12.tile([P, NT, S], BF16, tag="xl")
                wv = p12.tile([P, NT, D], BF16, tag="wv")

                wtiles = {}

                def fetch_w(et):
                    # et in 0..15: 0..7 = q cols, 8..15 = k cols
                    wh = wstr.tile([P, NT, P], BF16, tag="wh")
                    wl = wstr.tile([P, NT, P], BF16, tag="wl")
                    esl = slice(et * P, (et + 1) * P)
                    nc.sync.dma_start(
                        wh[:], wqkh_d[:, esl].rearrange("(t p) e -> p t e", p=P))
                    nc.sync.dma_start(
                        wl[:], wqkl_d[:, esl].rearrange("(t p) e -> p t e", p=P))
                    wtiles[et] = (wh, wl)

                # DMA order: r0 weights, x, wv, then attention residents
                fetch_w(0)
                nc.sync.dma_start(xh[:, 0, :], xh_d[0:P, :])
                nc.sync.dma_start(xl[:, 0, :], xl_d[0:P, :])
                fetch_w(8)
                fetch_w(1)
                fetch_w(9)
                for k in range(1, NT):
                    nc.sync.dma_start(xh[:, k, :], xh_d[k * P:(k + 1) * P, :])
                    nc.sync.dma_start(xl[:, k, :], xl_d[k * P:(k + 1) * P, :])
                for k in range(NT):
                    nc.sync.dma_start(wv[:, k, :], wv_d[k * P:(k + 1) * P, :])
                nc.sync.dma_start(mscl[:], m_d.ap().rearrange("(t p) j -> p t j", p=P))
                nc.sync.dma_start(ident[:], id_d[:])
                wptA = p12.tile([P, 4, D], BF16, tag="wptA")
                for ot in range(4):
                    nc.sync.dma_start(wptA[:, ot, :], wp_d[ot * P:(ot + 1) * P, :])

                qk_hist = {}

                def proj_qk(r):
                    # project q-cols et=r and k-cols et=8+r into rolling tiles
                    qh_r = qkroll.tile([P, 2, S], BF16, tag="qkhr")
                    ql_r = qkroll.tile([P, 2, S], BF16, tag="qklr")
                    qk_hist[r] = (qh_r, ql_r)
                    for side, et in ((0, r), (1, 8 + r)):
                        wh, wl = wtiles.pop(et)
                        esl = slice(et * P, (et + 1) * P)
                        for nh in range(2):
                            hsl = slice(nh * 512, (nh + 1) * 512)
                            ps = psA.tile([P, 512], F32, tag="ps")
                            ops = [(wt, xt, k) for k in range(NT)
                                   for (wt, xt) in ((wh, xh), (wl, xh), (wh, xl))]
                            for i, (wt, xt, k) in enumerate(ops):
                                nc.tensor.matmul(
                                    ps[:], wt[:, k, :], xt[:, k, hsl],
                                    start=(i == 0),
                                    stop=(i == len(ops) - 1) and not with_bias)
                            if with_bias:
                                nc.tensor.matmul(
                                    ps[:], biases["bqkh"][:, esl], ones_row[:, hsl],
                                    start=False, stop=False)
                                nc.tensor.matmul(
                                    ps[:], biases["bqkl"][:, esl], ones_row[:, hsl],
                                    start=False, stop=True)
                            nc.scalar.copy(qh_r[:, side, hsl], ps[:])
                            nc.vector.tensor_sub(
                                ql_r[:, side, hsl], ps[:], qh_r[:, side, hsl])

                def attention_pair(r):
                    qh_r, ql_r = qk_hist.pop(r)
                    for t in (2 * r, 2 * r + 1):
                        off = 64 * (t % 2)
                        qpk = stg.tile([P, S], BF16, tag="qpk")
                        khh = stg.tile([P, S], BF16, tag="khh")
                        kll = stg.tile([P, S], BF16, tag="kll")
                        nc.sync.dma_start(qpk[0:64, :], qh_r[off:off + 64, 0, :])
                        nc.sync.dma_start(qpk[64:128, :], ql_r[off:off + 64, 0, :])
                        nc.sync.dma_start(khh[0:64, :], qh_r[off:off + 64, 1, :])
                        nc.sync.dma_start(khh[64:128, :], qh_r[off:off + 64, 1, :])
                        nc.sync.dma_start(kll[0:64, :], ql_r[off:off + 64, 1, :])
                        nc.sync.dma_start(kll[64:128, :], ql_r[off:off + 64, 1, :])

                        ptb = ptpool.tile([P, NT, S], BF16, tag="ptb")
                        for it in range(NT):
                            isl = slice(it * P, (it + 1) * P)
                            ut = ppool.tile([P, S], F32, tag="ut")
                            raw = ppool.tile([P, 512], F32, tag="raw")
                            umin = ppool.tile([P, 1], F32, tag="umin")
                            for nh in range(2):
                                hsl = slice(nh * 512, (nh + 1) * 512)
                                pss = ps_s.tile([P, 512], F32, tag="pss")
                                nc.tensor.matmul(
                                    pss[:], qpk[:, isl], khh[:, hsl],
                                    start=True, stop=False)
                                nc.tensor.matmul(
                                    pss[:], qpk[:, isl], kll[:, hsl],
                                    start=False, stop=True)
                                if nh == 0:
                                    nc.vector.tensor_mul(
                                        ut[:, hsl], pss[:], mscl[:, it, hsl])
                                else:
                                    nc.scalar.copy(raw[:], pss[:])
                                    nc.gpsimd.tensor_mul(
                                        ut[:, hsl], raw[:], mscl[:, it, hsl])
                            nc.vector.tensor_reduce(
                                out=umin[:], in_=ut[:],
                                axis=mybir.AxisListType.X, op=MIN)
                            pt = ppool.tile([P, S], BF16, tag="pt")
                            nc.scalar.activation(
                                out=pt[:], in_=ut[:],
                                func=mybir.ActivationFunctionType.Exp,
                                bias=umin[:], scale=-1.0)
                            for trh in range(2):
                                pstr = ps_tr.tile([P, 512], BF16, tag="pstr")
                                for jj in range(4):
                                    jt = trh * 4 + jj
                                    nc.tensor.transpose(
                                        pstr[:, jj * P: (jj + 1) * P],
                                        pt[:, jt * P: (jt + 1) * P],
                                        ident[:])
                                dst = ptb[:, trh * 4: trh * 4 + 4, isl]
                                if (it + trh) % 2 == 0:
                                    nc.vector.tensor_copy(dst, pstr[:].rearrange(
                                        "p (j i) -> p j i", j=4))
                                else:
                                    nc.scalar.copy(dst, pstr[:].rearrange(
                                        "p (j i) -> p j i", j=4))
                        csl = slice(t * 64, t * 64 + 64)
                        olo = 64 * (t % 2)
                        for nh in range(2):
                            hsl = slice(nh * 512, (nh + 1) * 512)
                            pso = ps_o.tile([64, 512], F32, tag="pso")
                            for jt in range(NT):
                                nc.tensor.matmul(
                                    pso[:],
                                    vmat[:, jt, csl],
                                    ptb[:, jt, hsl],
                                    start=(jt == 0), stop=(jt == NT - 1))
                            nc.vector.tensor_copy(
                                otm[olo:olo + 64, t // 2, hsl], pso[:])

                # head-pair 0 projection first (TE starts as soon as its
                # weights + first x chunks land), then V projection
                proj_qk(0)
                for st in range(NT):
                    ssl = slice(st * P, (st + 1) * P)
                    for nh in range(2):
                        hsl = slice(nh * 512, (nh + 1) * 512)
                        ps = psA.tile([P, 512], F32, tag="ps")
                        first = True
                        for k in range(NT):
                            nc.tensor.matmul(
                                ps[:], xh[:, k, ssl], wv[:, k, hsl],
                                start=first, stop=(k == NT - 1) and not with_bias)
                            first = False
                        if with_bias:
                            nc.tensor.matmul(
                                ps[:], ones_row[:, ssl], biases["bvh"][:, hsl],
                                start=False, stop=False)
                            nc.tensor.matmul(
                                ps[:], ones_row[:, ssl], biases["bvl"][:, hsl],
                                start=False, stop=True)
                        nc.scalar.copy(vmat[:, st, hsl], ps[:])

                # software-pipelined: proj(r) overlaps attention(r-1);
                # the ot 0..3 half of the output projection (heads 0..7,
                # ready after pair 3) fills TensorE's pipeline-drain window
                # between the last two attention pairs, parking partials in
                # DRAM (SBUF has no room for an accumulator).
                for r in range(1, 9):
                    if r < 8:
                        if r + 1 < 8:
                            fetch_w(r + 1)
                            fetch_w(8 + r + 1)
                        proj_qk(r)
                    attention_pair(r - 1)
                    if r == 7:
                        for st in range(NT):
                            ssl = slice(st * P, (st + 1) * P)
                            for nh in range(2):
                                hsl = slice(nh * 512, (nh + 1) * 512)
                                ps = psA.tile([P, 512], F32, tag="ps")
                                for ot in range(4):
                                    nc.tensor.matmul(
                                        ps[:], otm[:, ot, ssl], wptA[:, ot, hsl],
                                        start=(ot == 0), stop=(ot == 3))
                                yp = ppool.tile([P, 512], F32, tag="raw")
                                nc.vector.tensor_copy(yp[:], ps[:])
                                nc.sync.dma_start(yp_d[ssl, hsl], yp[:])

            # ---------------- output projection ----------------
            with tc.tile_pool(name="proj", bufs=1) as proj, \
                 tc.tile_pool(name="ypool", bufs=2) as ypool, \
                 tc.tile_pool(name="psB", bufs=4, space="PSUM") as psB:
                wpt = proj.tile([P, 4, D], BF16, tag="wp")
                for ot in range(4):
                    nc.sync.dma_start(
                        wpt[:, ot, :], wp_d[(4 + ot) * P:(5 + ot) * P, :])
                for st in range(NT):
                    ssl = slice(st * P, (st + 1) * P)
                    yt = ypool.tile([P, D], F32, tag="yt")
                    ypt = ypool.tile([P, D], F32, tag="ypt")
                    nc.sync.dma_start(ypt[:], yp_d[ssl, :])
                    for nh in range(2):
                        hsl = slice(nh * 512, (nh + 1) * 512)
                        ps = psB.tile([P, 512], F32, tag="ps")
                        first = True
                        for ot in range(4):
                            nc.tensor.matmul(
                                ps[:], otm[:, 4 + ot, ssl], wpt[:, ot, hsl],
                                start=first, stop=(ot == 3) and not with_bias)
                            first = False
                        if with_bias:
                            nc.tensor.matmul(
                                ps[:], ones_row[:, ssl], biases["bph"][:, hsl],
                                start=False, stop=False)
                            nc.tensor.matmul(
                                ps[:], ones_row[:, ssl], biases["bpl"][:, hsl],
                                start=False, stop=True)
                        nc.vector.tensor_add(yt[:, hsl], ps[:], ypt[:, hsl])
                    nc.sync.dma_start(y_d[st * P: (st + 1) * P, :], yt[:])

    nc.compile()
    return nc


def _prep_inputs(x, mask, W_attn, b_attn, W_proj, b_proj, with_bias):
    x = np.asarray(x, np.float32)
    mask = np.asarray(mask, np.float32)
    W_attn = np.asarray(W_attn, np.float32)
    b_attn = np.asarray(b_attn, np.float32).reshape(-1)
    W_proj = np.asarray(W_proj, np.float32)
    b_proj = np.asarray(b_proj, np.float32).reshape(-1)

    wqk = W_attn[:, : 2 * D]
    wqkh = _bf16(wqk)
    wqkl = _bf16(wqk - wqkh.astype(np.float32))
    wv = _bf16(W_attn[:, 2 * D:])
    wp = _bf16(W_proj)

    shared = dict(wqkh=wqkh, wqkl=wqkl, wv=wv, wp=wp,
                  ident=_bf16(np.eye(P, dtype=np.float32)))
    if with_bias:
        def split_row(v):
            r = v.reshape(1, -1)
            h = _bf16(r)
            l = _bf16(r - h.astype(np.float32))
            return h, l

        shared["bqkh"], shared["bqkl"] = split_row(b_attn[: 2 * D])
        shared["bvh"], shared["bvl"] = split_row(b_attn[2 * D:])
        shared["bph"], shared["bpl"] = split_row(b_proj)

    in_maps = []
    for b in range(B):
        xT = np.ascontiguousarray(x[b].T)
        xh = _bf16(xT)
        xli = _bf16(xT - xh.astype(np.float32))
        in_maps.append(dict(xh=xh, xl=xli,
                            m=_bf16(mask[b, 0] * np.float32(SCALE)), **shared))
    return in_maps


def kernel(x, mask, W_attn, b_attn, W_proj, b_proj, _trace=False, _trace_kwargs=None):
    with_bias = bool(
        np.any(np.asarray(b_attn, np.float32)) or np.any(np.asarray(b_proj, np.float32))
    )
    key = ("nc", with_bias)
    if key not in _CACHE:
        _CACHE[key] = _build(with_bias)
    nc = _CACHE[key]
    in_maps = _prep_inputs(x, mask, W_attn, b_attn, W_proj, b_proj, with_bias)
    kw = {}
    if _trace:
        kw = dict(trace=True, **(_trace_kwargs or {}))
    res = run_bass_kernel_spmd(nc, in_maps, core_ids=list(range(B)), **kw)
    out = np.stack([res.results[b]["y"] for b in range(B)], axis=0)
    if _trace:
        _CACHE["last_results"] = res
    return out


# revision 46
# speedup vs baseline: 1.0415x; 1.0415x over previous
"""Multi-head attention (degenerate multiplicative-mask softmax) on 8 TRN2 cores.

Sharding: pure data-parallel over batch (B=8 -> 1 batch element per core).
No collectives. Each core computes its batch's full attention + output proj.

Structure (vs the 761us baseline; now fused-pipeline):
  - Projection and attention are INTERLEAVED per head-pair: iteration r
    projects q/k columns for head pair r while running the softmax-select
    pipeline (DVE/scalar/gpsimd) of head pair r-1. TensorE stays busy with
    proj matmuls while the other engines chew on selection, instead of two
    serial phases. q/k live in a rolling 3-deep pool (saves 6.5MB SBUF so
    x can stay resident).
  - QK^T: packed hi/lo 2-pass: contraction holds [qh(64); ql(64)] against
    [kh; kh] then [kl; kl] -> exact 4-term (qh+ql)(kh+kl) in 2 full-util
    matmuls. Operands DMA-staged per head (shift/duplication for free).
  - Mask-multiply split: half on DVE straight from PSUM, half via scalar
    drain + gpsimd tensor_tensor on a host-prescaled {0, 1.25e8} bf16 mask.
    DVE does the row-min reduce; exp(umin - u) on scalar; P@V drains on DVE.
    (tensor_tensor_reduce would fuse mask+min but crashes the device -
    NRT_EXEC_UNIT_UNRECOVERABLE, verified in isolation. gpsimd tensor_scalar
    runs in ~15us slow ucode; only plain tensor_tensor is fast there.)
  - Bias matmuls elided when biases are all-zero (the spec fills zeros);
    generic all-bias path kept as a fallback build.
Precision (validated in CPU sim vs f32 reference: 2 argmin flips/131072,
rel err 0.006): 3-pass bf16 hi/lo projections, exact packed QK^T on the
bf16-pair q/k, single-pass bf16 V/output path.
"""
import sys

sys.path.insert(0, "/opt/trn_rl_repo")

import numpy as np
import ml_dtypes

import concourse.bass as bass
import concourse.tile as tile
from concourse import bacc, mybir
from concourse.bass_utils import run_bass_kernel_spmd

F32 = mybir.dt.float32
BF16 = mybir.dt.bfloat16
MULT = mybir.AluOpType.mult
MIN = mybir.AluOpType.min

B, S, D = 8, 1024, 1024
H, DH = 16, 64
P = 128
NT = S // P
SCALE = 1.25e8  # 1e9 / 8

_CACHE = {}


def _bf16(a):
    return np.ascontiguousarray(a.astype(ml_dtypes.bfloat16))


def _build(with_bias=False):
    nc = bacc.Bacc(None)

    xh_d = nc.dram_tensor("xh", [D, S], BF16, kind="ExternalInput")  # x[b].T hi
    xl_d = nc.dram_tensor("xl", [D, S], BF16, kind="ExternalInput")  # x[b].T lo
    # mask pre-scaled by host to {0, SCALE} (both exact in bf16)
    m_d = nc.dram_tensor("m", [S, S], BF16, kind="ExternalInput")
    wqkh_d = nc.dram_tensor("wqkh", [D, 2 * D], BF16, kind="ExternalInput")
    wqkl_d = nc.dram_tensor("wqkl", [D, 2 * D], BF16, kind="ExternalInput")
    wv_d = nc.dram_tensor("wv", [D, D], BF16, kind="ExternalInput")
    wp_d = nc.dram_tensor("wp", [D, D], BF16, kind="ExternalInput")
    id_d = nc.dram_tensor("ident", [P, P], BF16, kind="ExternalInput")
    y_d = nc.dram_tensor("y", [S, D], F32, kind="ExternalOutput")
    if with_bias:
        bqkh_d = nc.dram_tensor("bqkh", [1, 2 * D], BF16, kind="ExternalInput")
        bqkl_d = nc.dram_tensor("bqkl", [1, 2 * D], BF16, kind="ExternalInput")
        bvh_d = nc.dram_tensor("bvh", [1, D], BF16, kind="ExternalInput")
        bvl_d = nc.dram_tensor("bvl", [1, D], BF16, kind="ExternalInput")
        bph_d = nc.dram_tensor("bph", [1, D], BF16, kind="ExternalInput")
        bpl_d = nc.dram_tensor("bpl", [1, D], BF16, kind="ExternalInput")

    with tile.TileContext(nc) as tc:
        with (
            tc.tile_pool(name="res", bufs=1) as res,
            tc.tile_pool(name="vres", bufs=1) as vres,
        ):
            mscl = res.tile([P, NT, S], BF16, tag="mscl")  # [i_sub, i_tile, j]
            otm = res.tile([P, NT, S], BF16, tag="otm")  # OT: [o_sub, o_tile, s]
            ident = res.tile([P, P], BF16, tag="ident")
            biases = {}
            if with_bias:
                ones_row = res.tile([1, S], BF16, tag="ones")
                nc.vector.memset(ones_row[:], 1.0)
                for nm, dd in (("bqkh", bqkh_d), ("bqkl", bqkl_d), ("bvh", bvh_d),
                               ("bvl", bvl_d), ("bph", bph_d), ("bpl", bpl_d)):
                    t = res.tile([1, dd.shape[1]], BF16, tag=nm)
                    nc.sync.dma_start(t[:], dd[:])
                    biases[nm] = t
            vmat = vres.tile([P, NT, D], BF16, tag="vmat")  # [j_sub, j_tile, c]

            # ------------- fused projection + attention -------------
            with tc.tile_pool(name="p12", bufs=1) as p12, \
                 tc.tile_pool(name="wstr", bufs=3) as wstr, \
                 tc.tile_pool(name="qkroll", bufs=3) as qkroll, \
                 tc.tile_pool(name="stg", bufs=3) as stg, \
                 tc.tile_pool(name="ppool", bufs=3) as ppool, \
                 tc.tile_pool(name="ptpool", bufs=2) as ptpool, \
                 tc.tile_pool(name="psA", bufs=2, space="PSUM") as psA, \
                 tc.tile_pool(name="ps_s", bufs=2, space="PSUM") as ps_s, \
                 tc.tile_pool(name="ps_tr", bufs=2, space="PSUM") as ps_tr, \
                 tc.tile_pool(name="ps_o", bufs=2, space="PSUM") as ps_o:
                xh = p12.tile([P, NT, S], BF16, tag="xh")  # [d_sub, d_tile, s]
                xl = p12.tile([P, NT, S], BF16, tag="xl")
                wv = p12.tile([P, NT, D], BF16, tag="wv")

                wtiles = {}

                def fetch_w(et):
                    # et in 0..15: 0..7 = q cols, 8..15 = k cols
                    wh = wstr.tile([P, NT, P], BF16, tag="wh")
                    wl = wstr.tile([P, NT, P], BF16, tag="wl")
                    esl = slice(et * P, (et + 1) * P)
                    nc.sync.dma_start(
                        wh[:], wqkh_d[:, esl].rearrange("(t p) e -> p t e", p=P))
                    nc.sync.dma_start(
                        wl[:], wqkl_d[:, esl].rearrange("(t p) e -> p t e", p=P))
                    wtiles[et] = (wh, wl)

                # DMA order: r0 weights, x, wv, then attention residents
                fetch_w(0)
                nc.sync.dma_start(xh[:, 0, :], xh_d[0:P, :])
                nc.sync.dma_start(xl[:, 0, :], xl_d[0:P, :])
                fetch_w(8)
                fetch_w(1)
                fetch_w(9)
                for k in range(1, NT):
                    nc.sync.dma_start(xh[:, k, :], xh_d[k * P:(k + 1) * P, :])
                    nc.sync.dma_start(xl[:, k, :], xl_d[k * P:(k + 1) * P, :])
                for k in range(NT):
                    nc.sync.dma_start(wv[:, k, :], wv_d[k * P:(k + 1) * P, :])
                nc.sync.dma_start(mscl[:], m_d.ap().rearrange("(t p) j -> p t j", p=P))
                nc.sync.dma_start(ident[:], id_d[:])

                qk_hist = {}

                def proj_qk(r):
                    # project q-cols et=r and k-cols et=8+r into rolling tiles
                    qh_r = qkroll.tile([P, 2, S], BF16, tag="qkhr")
                    ql_r = qkroll.tile([P, 2, S], BF16, tag="qklr")
                    qk_hist[r] = (qh_r, ql_r)
                    for side, et in ((0, r), (1, 8 + r)):
                        wh, wl = wtiles.pop(et)
                        esl = slice(et * P, (et + 1) * P)
                        for nh in range(2):
                            hsl = slice(nh * 512, (nh + 1) * 512)
                            ps = psA.tile([P, 512], F32, tag="ps")
                            ops = [(wt, xt, k) for k in range(NT)
                                   for (wt, xt) in ((wh, xh), (wl, xh), (wh, xl))]
                            for i, (wt, xt, k) in enumerate(ops):
                                nc.tensor.matmul(
                                    ps[:], wt[:, k, :], xt[:, k, hsl],
                                    start=(i == 0),
                                    stop=(i == len(ops) - 1) and not with_bias)
                            if with_bias:
                                nc.tensor.matmul(
                                    ps[:], biases["bqkh"][:, esl], ones_row[:, hsl],
                                    start=False, stop=False)
                                nc.tensor.matmul(
                                    ps[:], biases["bqkl"][:, esl], ones_row[:, hsl],
                                    start=False, stop=True)
                            nc.scalar.copy(qh_r[:, side, hsl], ps[:])
                            nc.vector.tensor_sub(
                                ql_r[:, side, hsl], ps[:], qh_r[:, side, hsl])

                def attention_pair(r):
                    qh_r, ql_r = qk_hist.pop(r)
                    for t in (2 * r, 2 * r + 1):
                        off = 64 * (t % 2)
                        qpk = stg.tile([P, S], BF16, tag="qpk")
                        khh = stg.tile([P, S], BF16, tag="khh")
                        kll = stg.tile([P, S], BF16, tag="kll")
                        nc.sync.dma_start(qpk[0:64, :], qh_r[off:off + 64, 0, :])
                        nc.sync.dma_start(qpk[64:128, :], ql_r[off:off + 64, 0, :])
                        nc.sync.dma_start(khh[0:64, :], qh_r[off:off + 64, 1, :])
                        nc.sync.dma_start(khh[64:128, :], qh_r[off:off + 64, 1, :])
                        nc.sync.dma_start(kll[0:64, :], ql_r[off:off + 64, 1, :])
                        nc.sync.dma_start(kll[64:128, :], ql_r[off:off + 64, 1, :])

                        ptb = ptpool.tile([P, NT, S], BF16, tag="ptb")
                        for it in range(NT):
                            isl = slice(it * P, (it + 1) * P)
                            ut = ppool.tile([P, S], F32, tag="ut")
                            raw = ppool.tile([P, 512], F32, tag="raw")
                            umin = ppool.tile([P, 1], F32, tag="umin")
                            for nh in range(2):
                                hsl = slice(nh * 512, (nh + 1) * 512)
                                pss = ps_s.tile([P, 512], F32, tag="pss")
                                nc.tensor.matmul(
                                    pss[:], qpk[:, isl], khh[:, hsl],
                                    start=True, stop=False)
                                nc.tensor.matmul(
                                    pss[:], qpk[:, isl], kll[:, hsl],
                                    start=False, stop=True)
                                if nh == 0:
                                    nc.vector.tensor_mul(
                                        ut[:, hsl], pss[:], mscl[:, it, hsl])
                                else:
                                    nc.scalar.copy(raw[:], pss[:])
                                    nc.gpsimd.tensor_mul(
                                        ut[:, hsl], raw[:], mscl[:, it, hsl])
                            nc.vector.tensor_reduce(
                                out=umin[:], in_=ut[:],
                                axis=mybir.AxisListType.X, op=MIN)
                            pt = ppool.tile([P, S], BF16, tag="pt")
                            nc.scalar.activation(
                                out=pt[:], in_=ut[:],
                                func=mybir.ActivationFunctionType.Exp,
                                bias=umin[:], scale=-1.0)
                            for trh in range(2):
                                pstr = ps_tr.tile([P, 512], BF16, tag="pstr")
                                for jj in range(4):
                                    jt = trh * 4 + jj
                                    nc.tensor.transpose(
                                        pstr[:, jj * P: (jj + 1) * P],
                                        pt[:, jt * P: (jt + 1) * P],
                                        ident[:])
                                dst = ptb[:, trh * 4: trh * 4 + 4, isl]
                                if (it + trh) % 2 == 0:
                                    nc.vector.tensor_copy(dst, pstr[:].rearrange(
                                        "p (j i) -> p j i", j=4))
                                else:
                                    nc.scalar.copy(dst, pstr[:].rearrange(
                                        "p (j i) -> p j i", j=4))
                        csl = slice(t * 64, t * 64 + 64)
                        olo = 64 * (t % 2)
                        for nh in range(2):
                            hsl = slice(nh * 512, (nh + 1) * 512)
                            pso = ps_o.tile([64, 512], F32, tag="pso")
                            for jt in range(NT):
                                nc.tensor.matmul(
                                    pso[:],
                                    vmat[:, jt, csl],
                                    ptb[:, jt, hsl],
                                    start=(jt == 0), stop=(jt == NT - 1))
                            nc.vector.tensor_copy(
                                otm[olo:olo + 64, t // 2, hsl], pso[:])

                # head-pair 0 projection first (TE starts as soon as its
                # weights + first x chunks land), then V projection
                proj_qk(0)
                for st in range(NT):
                    ssl = slice(st * P, (st + 1) * P)
                    for nh in range(2):
                        hsl = slice(nh * 512, (nh + 1) * 512)
                        ps = psA.tile([P, 512], F32, tag="ps")
                        first = True
                        for k in range(NT):
                            nc.tensor.matmul(
                                ps[:], xh[:, k, ssl], wv[:, k, hsl],
                                start=first, stop=(k == NT - 1) and not with_bias)
                            first = False
                        if with_bias:
                            nc.tensor.matmul(
                                ps[:], ones_row[:, ssl], biases["bvh"][:, hsl],
                                start=False, stop=False)
                            nc.tensor.matmul(
                                ps[:], ones_row[:, ssl], biases["bvl"][:, hsl],
                                start=False, stop=True)
                        nc.scalar.copy(vmat[:, st, hsl], ps[:])

                # software-pipelined: proj(r) overlaps attention(r-1)
                for r in range(1, 9):
                    if r < 8:
                        if r + 1 < 8:
                            fetch_w(r + 1)
                            fetch_w(8 + r + 1)
                        proj_qk(r)
                    attention_pair(r - 1)

            # ---------------- output projection ----------------
            with tc.tile_pool(name="proj", bufs=1) as proj, \
                 tc.tile_pool(name="ypool", bufs=2) as ypool, \
                 tc.tile_pool(name="psB", bufs=4, space="PSUM") as psB:
                wpt = proj.tile([P, NT, D], BF16, tag="wp")
                for ot in range(NT):
                    nc.sync.dma_start(wpt[:, ot, :], wp_d[ot * P:(ot + 1) * P, :])
                for st in range(NT):
                    ssl = slice(st * P, (st + 1) * P)
                    yt = ypool.tile([P, D], F32, tag="yt")
                    for nh in range(2):
                        hsl = slice(nh * 512, (nh + 1) * 512)
                        ps = psB.tile([P, 512], F32, tag="ps")
                        first = True
                        for ot in range(NT):
                            nc.tensor.matmul(
                                ps[:], otm[:, ot, ssl], wpt[:, ot, hsl],
                                start=first, stop=(ot == NT - 1) and not with_bias)
                            first = False
                        if with_bias:
                            nc.tensor.matmul(
                                ps[:], ones_row[:, ssl], biases["bph"][:, hsl],
                                start=False, stop=False)
                            nc.tensor.matmul(
                                ps[:], ones_row[:, ssl], biases["bpl"][:, hsl],
                                start=False, stop=True)
                        nc.scalar.copy(yt[:, hsl], ps[:])
                    nc.sync.dma_start(y_d[st * P: (st + 1) * P, :], yt[:])

    nc.compile()
    return nc


def _prep_inputs(x, mask, W_attn, b_attn, W_proj, b_proj, with_bias):
    x = np.asarray(x, np.float32)
    mask = np.asarray(mask, np.float32)
    W_attn = np.asarray(W_attn, np.float32)
    b_attn = np.asarray(b_attn, np.float32).reshape(-1)
    W_proj = np.asarray(W_proj, np.float32)
    b_proj = np.asarray(b_proj, np.float32).reshape(-1)

    wqk = W_attn[:, : 2 * D]
    wqkh = _bf16(wqk)
    wqkl = _bf16(wqk - wqkh.astype(np.float32))
    wv = _bf16(W_attn[:, 2 * D:])
    wp = _bf16(W_proj)

    shared = dict(wqkh=wqkh, wqkl=wqkl, wv=wv, wp=wp,
                  ident=_bf16(np.eye(P, dtype=np.float32)))
    if with_bias:
        def split_row(v):
            r = v.reshape(1, -1)
            h = _bf16(r)
            l = _bf16(r - h.astype(np.float32))
            return h, l

        shared["bqkh"], shared["bqkl"] = split_row(b_attn[: 2 * D])
        shared["bvh"], shared["bvl"] = split_row(b_attn[2 * D:])
        shared["bph"], shared["bpl"] = split_row(b_proj)

    in_maps = []
    for b in range(B):
        xT = np.ascontiguousarray(x[b].T)
        xh = _bf16(xT)
        xli = _bf16(xT - xh.astype(np.float32))
        in_maps.append(dict(xh=xh, xl=xli,
                            m=_bf16(mask[b, 0] * np.float32(SCALE)), **shared))
    return in_maps


def kernel(x, mask, W_attn, b_attn, W_proj, b_proj, _trace=False, _trace_kwargs=None):
    with_bias = bool(
        np.any(np.asarray(b_attn, np.float32)) or np.any(np.asarray(b_proj, np.float32))
    )
    key = ("nc", with_bias)
    if key not in _CACHE:
        _CACHE[key] = _build(with_bias)
    nc = _CACHE[key]
    in_maps = _prep_inputs(x, mask, W_attn, b_attn, W_proj, b_proj, with_bias)
    kw = {}
    if _trace:
        kw = dict(trace=True, **(_trace_kwargs or {}))
    res = run_bass_kernel_spmd(nc, in_maps, core_ids=list(range(B)), **kw)
    out = np.stack([res.results[b]["y"] for b in range(B)], axis=0)
    if _trace:
        _CACHE["last_results"] = res
    return out
